# revision 2
# baseline (speedup 1.0000x reference)
"""BandSplit Trainium2 kernel, v2 (bf16 datapath).

Math (per sample b, band j covering flat-channel segment [q0, q0+w)):
  x viewed as (T, 962); GroupNorm over (T, w) per band, then per-band
  1x1 conv:  out_j = fw_j @ xn_j^T + fb_j.

Folding: with A_k = rstd_j*nw_k and B_k = nb_k - mu_j*A_k,
  out[c,t] = rstd_j * (fwn_j @ x)[c,t] + (c1[c,j] - mu_j*rstd_j*c2[c,j])
  where fwn = fw*nw (HOST, bf16), c1 = fb + fw@nb, c2 = fw@nw (HOST, f32).
Only mu_j, rstd_j are computed on device, so the output matmuls use
host-constant weights and normalization enters via a fused scale+bias
in the PSUM->staging copy.

Schedule (one sample per core, 8 cores data-parallel):
  L: x (bf16) in 4 batched DMAs of 4 t-tiles; PE-transpose each 125-t
     tile into 10 band-aligned resident xT chunks (PSUM banks pack
     4 chunks x 2 t-tiles); bn_stats per (chunk, 500-t window) on DVE.
  S: bn_aggr -> per-row (mean, var) -> (s1, s2) -> indicator matmul ->
     per-band mu, rstd; broadcast to channel partitions via a diag
     matmul; btot = c1 - (mu*rstd)*c2.
  O: per (250-t chunk, band) bf16 matmul K=w into PSUM; ACT/DVE fused
     copy out = psum*rstd_j + btot[:,j] into band-interleaved staging;
     one DMA per t-chunk writes (C, 250, 34) contiguously.

All constants ship as one packed (128, NCONST) f32 tensor -> a single
DMA (every DMA pays ~650ns on the shared hardware DGE).
"""
import numpy as np

GROUPS = [(0, 1, 5), (5, 19, 4), (81, 6, 10), (141, 7, 40), (421, 1, 60)]
B, C, T, Q, NB = 8, 128, 2000, 962, 34
EPS = 1e-5
# TT even: bf16 PSUM slot widths/offsets must stay 32-bit-word aligned
TT, NLD = 100, 20     # load t-tiles
LDBS = [1, 1, 2, 4, 4, 4, 4]      # t-tiles per x DMA (fast ramp)
assert sum(LDBS) == NLD
TCS = [100, 200, 500, 500, 500, 200]   # output t-chunks (fast ramp + drain)
assert sum(TCS) == T
TBLK = 2              # t-tiles per transpose-PSUM tile / xT copy
TTP = TT              # PSUM slot width (no pad needed, TT even)
# bn_stats windows (start, len): short tail windows so the last stats
# work starts before the final t-tile lands; bn_aggr count-weights them
WINS = [(0, 500), (500, 500), (1000, 500), (1500, 300), (1800, 200)]

BANDS = []
for _g, (_off, _n, _s) in enumerate(GROUPS):
    for _i in range(_n):
        BANDS.append((2 * _off + _i * 2 * _s, 2 * _s, _g, _i))
assert len(BANDS) == NB and BANDS[-1][0] + BANDS[-1][1] == Q

# band-aligned chunks: consecutive bands packed into <=128 rows
CHUNKS = []   # (q0, rows, [band indices])
_cur, _rows, _q0 = [], 0, 0
for _j, (_qb, _w, _g, _i) in enumerate(BANDS):
    if _rows + _w > 128:
        CHUNKS.append((_q0, _rows, _cur))
        _cur, _rows, _q0 = [], 0, _qb
    _cur.append(_j)
    _rows += _w
CHUNKS.append((_q0, _rows, _cur))
NCH = len(CHUNKS)
CHUNK_OF = {}
for _c, (_q0, _r, _bl) in enumerate(CHUNKS):
    for _j in _bl:
        CHUNK_OF[_j] = _c

# packed const layout (f32 columns)
_off = 0
CONST_OFF = {}
def _creserve(name, cols):
    global _off
    CONST_OFF[name] = _off
    _off += cols
_creserve("ident", 64)                    # (128,128) bf16
for _j in range(NB):
    _creserve(f"fwn{_j}", 64)             # (128,128) bf16 each
for _c in range(NCH):
    _creserve(f"ind{_c}", NB)             # (128,34) f32
_creserve("c1t", NB)
_creserve("c2t", NB)
_creserve("ones34", C)                    # rows 0..33 used
_creserve("eye34", NB)
_creserve("invc", 1)
_creserve("epsv", 1)
NCONST = _off


def _align(r0, r1):
    """Largest legal quadrant base <= r0 covering [r0, r1)."""
    for base in (64, 32, 0):
        if base > r0:
            continue
        K = r1 - base
        if K <= 32:
            return base, K
        if K <= 64 and base in (0, 64):
            return base, K
        if base == 0:
            return 0, K
    raise AssertionError((r0, r1))


def _band_window(j):
    q0, w, _g, _i = BANDS[j]
    c = CHUNK_OF[j]
    r0 = q0 - CHUNKS[c][0]
    base, K = _align(r0, r0 + w)
    return c, base, K


def host_constants(inputs):
    import concourse.mybir as mybir
    bf16 = mybir.dt.np(mybir.dt.bfloat16)

    fws = [np.asarray(inputs[f"fw{g}"], np.float32) for g in range(5)]
    nws = [np.asarray(inputs[f"nw{g}"], np.float32) for g in range(5)]
    nbs = [np.asarray(inputs[f"nb{g}"], np.float32) for g in range(5)]
    fbs = [np.asarray(inputs[f"fb{g}"], np.float32) for g in range(5)]

    cst = np.zeros((128, NCONST), np.float32)

    def put_bf16(name, arr128x128):
        u16 = np.ascontiguousarray(arr128x128.astype(bf16)).view(np.uint16)
        u32 = u16[:, 0::2].astype(np.uint32) | (
            u16[:, 1::2].astype(np.uint32) << 16)
        cst[:, CONST_OFF[name]:CONST_OFF[name] + 64] = u32.view(np.float32)

    put_bf16("ident", np.eye(128, dtype=np.float32))
    c1t = np.zeros((C, NB), np.float32)
    c2t = np.zeros((C, NB), np.float32)
    for j, (q0, w, g, i) in enumerate(BANDS):
        c = CHUNK_OF[j]
        r0 = q0 - CHUNKS[c][0]
        fw, nw, nb, fb = fws[g][i], nws[g][i], nbs[g][i], fbs[g][i]
        fwn = np.zeros((128, C), np.float32)
        fwn[r0:r0 + w, :] = (fw * nw[None, :]).T
        put_bf16(f"fwn{j}", fwn)
        c1t[:, j] = fb + fw @ nb
        c2t[:, j] = fw @ nw
    for c, (q0, rows, bl) in enumerate(CHUNKS):
        ind = np.zeros((128, NB), np.float32)
        for j in bl:
            qb, w, _g, _i = BANDS[j]
            ind[qb - q0:qb - q0 + w, j] = 1.0
        cst[:, CONST_OFF[f"ind{c}"]:CONST_OFF[f"ind{c}"] + NB] = ind
    cst[:, CONST_OFF["c1t"]:CONST_OFF["c1t"] + NB] = c1t
    cst[:, CONST_OFF["c2t"]:CONST_OFF["c2t"] + NB] = c2t
    cst[0:NB, CONST_OFF["ones34"]:CONST_OFF["ones34"] + C] = 1.0
    cst[0:NB, CONST_OFF["eye34"]:CONST_OFF["eye34"] + NB] = np.eye(NB)
    cst[0:NB, CONST_OFF["invc"]] = [1.0 / w for (_q0, w, _g, _i) in BANDS]
    cst[0:NB, CONST_OFF["epsv"]] = EPS
    return {"cst": cst}


def build_module(phases=4, ntc_cap=99, skip_out_dma=False, out_mode="mix"):
    import concourse.bacc as bacc
    import concourse.tile as tile
    import concourse.mybir as mybir
    from contextlib import ExitStack

    f32 = mybir.dt.float32
    bf16 = mybir.dt.bfloat16
    AF = mybir.ActivationFunctionType
    ALU = mybir.AluOpType
    nc = bacc.Bacc(None)

    x_d = nc.declare_dram_parameter("x", [T, Q], bf16, isOutput=False)
    cst_d = nc.declare_dram_parameter("cst", [128, NCONST], f32, isOutput=False)
    out_d = nc.declare_dram_parameter("out", [C, T, NB], f32, isOutput=True)

    with tile.TileContext(nc) as tc, ExitStack() as ctx:
        cpool = ctx.enter_context(tc.tile_pool(name="cpool", bufs=1))
        ldpool = ctx.enter_context(tc.tile_pool(name="ld", bufs=2))
        stpool = ctx.enter_context(tc.tile_pool(name="st", bufs=2))
        smpool = ctx.enter_context(tc.tile_pool(name="sm", bufs=4))
        # PSUM: 8 banks of 2KB, bank-granular tiles. 4 transpose banks
        # (4 chunks x 2 t-tiles packed per bank), 3 output banks, 1 small.
        ps_tp = ctx.enter_context(tc.tile_pool(name="ps_tp", bufs=4, space="PSUM"))
        ps_out = ctx.enter_context(tc.tile_pool(name="ps_out", bufs=3, space="PSUM"))
        ps_sm = ctx.enter_context(tc.tile_pool(name="ps_sm", bufs=1, space="PSUM"))

        # ident first (transposes need it); the bulk lands after the x DMAs
        cst = cpool.tile([128, NCONST], f32, tag="cst", name="cst_t")
        nc.sync.dma_start(cst[:, 0:64], cst_d[:, 0:64])

        def cview(name, cols, dt=f32, parts=128):
            o = CONST_OFF[name]
            v = cst[0:parts, o:o + (cols if dt is f32 else cols // 2)]
            return v.bitcast(dt) if dt is not f32 else v

        ident = cview("ident", 128, bf16)
        fwp = [cview(f"fwn{j}", 128, bf16) for j in range(NB)]
        ind = [cview(f"ind{c}", NB) for c in range(NCH)]
        c1t = cview("c1t", NB)
        c2t = cview("c2t", NB)
        ones34 = cview("ones34", C, parts=NB)
        eye34 = cview("eye34", NB, parts=NB)
        invc = cview("invc", 1, parts=NB)
        epsap = cview("epsv", 1, parts=NB)

        xT = [cpool.tile([CHUNKS[c][1], T], bf16, tag=f"xT{c}", name=f"xT{c}")
              for c in range(NCH)]
        musig = cpool.tile([NB, 2], f32, tag="musig", name="musig")
        bcast = cpool.tile([C, 2 * NB], f32, tag="bcast", name="bcast")
        btot = cpool.tile([C, NB], f32, tag="btot", name="btot")

        # preload both ACT tables used later (Sqrt for stats, Identity for
        # output copies) so the loads sit off the critical path
        warm = smpool.tile([1, 2], f32, tag="warm", name="warm")
        nc.vector.memset(warm[:], 0.0)
        nc.scalar.activation(warm[:, 0:1], warm[:, 0:1], AF.Sqrt,
                             bias=warm[:, 1:2], scale=1.0)
        nc.scalar.activation(warm[:, 0:1], warm[:, 0:1], AF.Identity,
                             bias=warm[:, 1:2], scale=1.0)

        # ---- L: load + transpose (+ windowed stats) ----
        CGRP = [[0, 1, 2, 3], [4, 5, 6, 7], [8, 9]]
        st6 = [smpool.tile([CHUNKS[c][1], 6 * len(WINS)], f32,
                           tag=f"st6_{c}", name=f"st6_{c}", bufs=1)
               for c in range(NCH)]
        tp = {}
        nat = None
        ld_starts = np.cumsum([0] + LDBS[:-1]).tolist()
        nat_off = 0
        for tt in range(NLD):
            blk, half = divmod(tt, TBLK)
            if tt in ld_starts:
                ldb = LDBS[ld_starts.index(tt)]
                nat_off = tt
                nat = ldpool.tile([TT, ldb, Q], bf16, tag="nat",
                                  name=f"nat{tt}")
                xv = x_d.rearrange("(b t) q -> t b q", t=TT)
                nc.sync.dma_start(nat[:], xv[:, tt:tt + ldb, :])
                if tt == 0:
                    nc.sync.dma_start(cst[:, 64:], cst_d[:, 64:])
                # tiny PE matmul absorbs the DMA-queue wait so transposes
                # carry at most one wait each
                dmy = ps_sm.tile([1, 1], f32, tag="small", name=f"dmy{tt}")
                nc.tensor.matmul(dmy[:], nat[0:1, 0, 0:1], nat[0:1, 0, 0:1],
                                 start=True, stop=True)
            if half == 0:
                for g, grp in enumerate(CGRP):
                    tp[g] = ps_tp.tile([128, len(grp) * TBLK * TTP], bf16,
                                       tag="tp", name=f"tp{blk}_{g}")
            for c in range(NCH):
                q0, rows, _bl = CHUNKS[c]
                g, idx = divmod(c, 4)
                col = (idx * TBLK + half) * TTP
                nc.tensor.transpose(tp[g][0:rows, col:col + TT],
                                    nat[:, tt - nat_off, q0:q0 + rows],
                                    ident[0:TT, 0:TT])
            if half == TBLK - 1:
                for c in range(NCH):
                    rows = CHUNKS[c][1]
                    g, idx = divmod(c, 4)
                    tpv = tp[g].rearrange("p (s t) -> p s t", t=TTP)
                    src = tpv[0:rows, idx * TBLK:(idx + 1) * TBLK, 0:TT]
                    dst = xT[c][:, blk * TBLK * TT:(blk + 1) * TBLK * TT
                                ].rearrange("p (s t) -> p s t", t=TT)
                    # DVE carries bn_stats too; give it fewer copies
                    if (c + blk) % 10 < 2:
                        nc.vector.tensor_copy(dst, src)
                    else:
                        nc.scalar.copy(dst, src)
            if phases >= 2:
                done_t = (tt + 1) * TT if half == TBLK - 1 else 0
                for wi, (ws, wl) in enumerate(WINS):
                    if done_t and done_t - TBLK * TT < ws + wl <= done_t:
                        assert ws % 2 == 0
                        for c in range(NCH):
                            rows = CHUNKS[c][1]
                            nc.vector.bn_stats(st6[c][:, 6 * wi:6 * (wi + 1)],
                                               xT[c][0:rows, ws:ws + wl])

        if phases == 1:
            nc.sync.dma_start(out_d[0:CHUNKS[0][1], 0:T // 2, 0],
                              xT[0][:].bitcast(f32))

        if phases >= 2:
            # ---- S: stats -> mu, rstd -> broadcast + btot ----
            stats_ps = ps_sm.tile([NB, 2], f32, tag="small", name="stats_ps")
            for c in range(NCH):
                rows = CHUNKS[c][1]
                s12 = smpool.tile([rows, 2], f32, tag="s12", name=f"s12_{c}")
                tmp = smpool.tile([rows, 1], f32, tag="tmp", name=f"tmp{c}")
                nc.vector.bn_aggr(s12[:], st6[c][:])
                nc.vector.tensor_mul(tmp[:], s12[:, 0:1], s12[:, 0:1])
                nc.vector.tensor_add(s12[:, 1:2], s12[:, 1:2], tmp[:])
                nc.tensor.matmul(stats_ps[:], ind[c][0:rows, :], s12[:],
                                 start=(c == 0), stop=(c == NCH - 1))

            # musig[:,0]=mu, musig[:,1]=rstd  (invc = 1/w, sums are of
            # per-row means/second-moments so /w gives band stats)
            ex2 = smpool.tile([NB, 1], f32, tag="ex2", name="ex2")
            var_t = smpool.tile([NB, 1], f32, tag="var", name="var_t")
            std_t = smpool.tile([NB, 1], f32, tag="std", name="std_t")
            nc.vector.tensor_scalar_mul(musig[:, 0:1], stats_ps[:, 0:1],
                                        invc[:])
            nc.vector.tensor_scalar_mul(ex2[:], stats_ps[:, 1:2], invc[:])
            nc.vector.tensor_mul(var_t[:], musig[:, 0:1], musig[:, 0:1])
            nc.vector.tensor_sub(var_t[:], ex2[:], var_t[:])
            nc.scalar.activation(std_t[:], var_t[:], AF.Sqrt, bias=epsap[:],
                                 scale=1.0)
            nc.vector.reciprocal(musig[:, 1:2], std_t[:])

            # diag trick: [diag(rstd) | diag(mu*rstd)], then ones34^T @ .
            mrs = smpool.tile([NB, 1], f32, tag="mrs", name="mrs")
            nc.vector.tensor_mul(mrs[:], musig[:, 0:1], musig[:, 1:2])
            dg = smpool.tile([NB, 2 * NB], f32, tag="dg", name="dg")
            nc.vector.tensor_scalar_mul(dg[:, 0:NB], eye34[:], musig[:, 1:2])
            nc.vector.tensor_scalar_mul(dg[:, NB:2 * NB], eye34[:], mrs[:])
            bc_ps = ps_sm.tile([C, 2 * NB], f32, tag="small", name="bc_ps")
            nc.tensor.matmul(bc_ps[:], ones34[:], dg[:], start=True, stop=True)
            nc.vector.tensor_copy(bcast[:], bc_ps[:])
            nc.vector.tensor_mul(btot[:], bcast[:, NB:2 * NB], c2t[:])
            nc.vector.tensor_sub(btot[:], c1t[:], btot[:])

        if phases == 2:
            nc.sync.dma_start(out_d[0:NB, 0, 0:2], musig[:])
            nc.sync.dma_start(out_d[0:C, 1, 0:NB], btot[:])

        if phases >= 3:
            # ---- O: per (t-chunk, band) matmul + fused scale/bias copy ----
            t0 = 0
            for tk, TC in enumerate(TCS[:min(len(TCS), ntc_cap)]):
                stag = stpool.tile([C, max(TCS) * NB], f32, tag="stag",
                                   name=f"stag{tk}")
                sv = stag.rearrange("p (t j) -> p t j", j=NB)
                for j in range(NB):
                    c, base, K = _band_window(j)
                    ops = ps_out.tile([C, TC], f32, tag="outp",
                                      name=f"ops{tk}_{j}")
                    nc.tensor.matmul(ops[:], fwp[j][base:base + K, :],
                                     xT[c][base:base + K, t0:t0 + TC],
                                     start=True, stop=True)
                    use_act = ((j + tk) % 2 < 1 if tk == 0
                               else (j + tk) % 9 < 5)
                    if out_mode == "plain":
                        (nc.scalar.copy if use_act else nc.vector.tensor_copy)(
                            sv[:, 0:TC, j], ops[:])
                    elif out_mode == "act" or (out_mode == "mix" and use_act):
                        nc.scalar.activation(sv[:, 0:TC, j], ops[:],
                                             AF.Identity,
                                             bias=btot[:, j:j + 1],
                                             scale=bcast[:, j:j + 1])
                    else:
                        nc.vector.tensor_scalar(sv[:, 0:TC, j], ops[:],
                                                bcast[:, j:j + 1],
                                                btot[:, j:j + 1],
                                                ALU.mult, ALU.add)
                if not skip_out_dma:
                    nc.sync.dma_start(out_d[:, t0:t0 + TC, :],
                                      sv[:, 0:TC, :])
                t0 += TC

    _finalize(nc)
    return nc


def _finalize(nc):
    import concourse.mybir as mybir
    nc.compile()
    # compile()'s late passes can leave >1-wait instructions, which walrus
    # rejects for some instruction types and hardware mishandles for others.
    nc.generate_event_semaphores()
    nc.codegen_inst_isa_subclasses()
    m2 = mybir.parse_bytes(nc.to_json_bytes())
    for fn in m2.functions:
        for bb in fn.blocks:
            for i in bb.instructions:
                si = i.sync_info
                n = len(si.on_wait) if si and si.on_wait else 0
                assert n <= 1 or type(i).__name__ == "InstEventSemaphore", (
                    f"multi-wait survived: {i.name} {type(i).__name__} {n}")


_CACHE = {}


def _get_module():
    if "nc" not in _CACHE:
        _CACHE["nc"] = build_module()
    return _CACHE["nc"]


def prepare_in_maps(inputs):
    import concourse.mybir as mybir
    bf16 = mybir.dt.np(mybir.dt.bfloat16)
    x = np.ascontiguousarray(
        np.asarray(inputs["x"], dtype=np.float32)).reshape(B, T, Q).astype(bf16)
    base = host_constants(inputs)
    return [dict(base, x=x[i]) for i in range(B)]


def kernel(**inputs):
    from concourse.bass_utils import run_bass_kernel_spmd

    nc = _get_module()
    in_maps = prepare_in_maps(inputs)
    res = run_bass_kernel_spmd(nc, in_maps, core_ids=list(range(B)))
    return np.stack([res.results[i]["out"] for i in range(B)], axis=0)


# revision 3
# speedup vs baseline: 1.1829x; 1.1829x over previous
"""BandSplit Trainium2 kernel, v2 (bf16 datapath).

Math (per sample b, band j covering flat-channel segment [q0, q0+w)):
  x viewed as (T, 962); GroupNorm over (T, w) per band, then per-band
  1x1 conv:  out_j = fw_j @ xn_j^T + fb_j.

Folding: with A_k = rstd_j*nw_k and B_k = nb_k - mu_j*A_k,
  out[c,t] = rstd_j * (fwn_j @ x)[c,t] + (c1[c,j] - mu_j*rstd_j*c2[c,j])
  where fwn = fw*nw (HOST, bf16), c1 = fb + fw@nb, c2 = fw@nw (HOST, f32).
Only mu_j, rstd_j are computed on device, so the output matmuls use
host-constant weights and normalization enters via a fused scale+bias
in the PSUM->staging copy.

Schedule (one sample per core, 8 cores data-parallel):
  L: x (bf16) in 4 batched DMAs of 4 t-tiles; PE-transpose each 125-t
     tile into 10 band-aligned resident xT chunks (PSUM banks pack
     4 chunks x 2 t-tiles); bn_stats per (chunk, 500-t window) on DVE.
  S: bn_aggr -> per-row (mean, var) -> (s1, s2) -> indicator matmul ->
     per-band mu, rstd; broadcast to channel partitions via a diag
     matmul; btot = c1 - (mu*rstd)*c2.
  O: per (250-t chunk, band) bf16 matmul K=w into PSUM; ACT/DVE fused
     copy out = psum*rstd_j + btot[:,j] into band-interleaved staging;
     one DMA per t-chunk writes (C, 250, 34) contiguously.

All constants ship as one packed (128, NCONST) f32 tensor -> a single
DMA (every DMA pays ~650ns on the shared hardware DGE).
"""
import numpy as np

GROUPS = [(0, 1, 5), (5, 19, 4), (81, 6, 10), (141, 7, 40), (421, 1, 60)]
B, C, T, Q, NB = 8, 128, 2000, 962, 34
EPS = 1e-5
# TT even: bf16 PSUM slot widths/offsets must stay 32-bit-word aligned
TT, NLD = 100, 20     # load t-tiles
LDBS = [1, 1, 2, 4, 4, 4, 4]      # t-tiles per x DMA (fast ramp)
assert sum(LDBS) == NLD
TCS = [100, 200, 300, 400, 500, 500]   # output t-chunks (graduated ramp)
assert sum(TCS) == T
TBLK = 2              # t-tiles per transpose-PSUM tile / xT copy
TTP = TT              # PSUM slot width (no pad needed, TT even)
# bn_stats windows (start, len): short tail windows so the last stats
# work starts before the final t-tile lands; bn_aggr count-weights them
WINS = [(0, 500), (500, 500), (1000, 500), (1500, 300), (1800, 200)]

BANDS = []
for _g, (_off, _n, _s) in enumerate(GROUPS):
    for _i in range(_n):
        BANDS.append((2 * _off + _i * 2 * _s, 2 * _s, _g, _i))
assert len(BANDS) == NB and BANDS[-1][0] + BANDS[-1][1] == Q

# band-aligned chunks: consecutive bands packed into <=128 rows
CHUNKS = []   # (q0, rows, [band indices])
_cur, _rows, _q0 = [], 0, 0
for _j, (_qb, _w, _g, _i) in enumerate(BANDS):
    if _rows + _w > 128:
        CHUNKS.append((_q0, _rows, _cur))
        _cur, _rows, _q0 = [], 0, _qb
    _cur.append(_j)
    _rows += _w
CHUNKS.append((_q0, _rows, _cur))
NCH = len(CHUNKS)
CHUNK_OF = {}
for _c, (_q0, _r, _bl) in enumerate(CHUNKS):
    for _j in _bl:
        CHUNK_OF[_j] = _c

# packed const layout (f32 columns)
_off = 0
CONST_OFF = {}
def _creserve(name, cols):
    global _off
    CONST_OFF[name] = _off
    _off += cols
_creserve("ident", 64)                    # (128,128) bf16
for _j in range(NB):
    _creserve(f"fwn{_j}", 64)             # (128,128) bf16 each
for _c in range(NCH):
    _creserve(f"ind{_c}", NB)             # (128,34) f32
_creserve("c1t", NB)
_creserve("c2t", NB)
_creserve("ones34", C)                    # rows 0..33 used
_creserve("eye34", NB)
_creserve("invc", 1)
_creserve("epsv", 1)
NCONST = _off


def _align(r0, r1):
    """Largest legal quadrant base <= r0 covering [r0, r1)."""
    for base in (64, 32, 0):
        if base > r0:
            continue
        K = r1 - base
        if K <= 32:
            return base, K
        if K <= 64 and base in (0, 64):
            return base, K
        if base == 0:
            return 0, K
    raise AssertionError((r0, r1))


def _band_window(j):
    q0, w, _g, _i = BANDS[j]
    c = CHUNK_OF[j]
    r0 = q0 - CHUNKS[c][0]
    base, K = _align(r0, r0 + w)
    return c, base, K


def host_constants(inputs):
    import concourse.mybir as mybir
    bf16 = mybir.dt.np(mybir.dt.bfloat16)

    fws = [np.asarray(inputs[f"fw{g}"], np.float32) for g in range(5)]
    nws = [np.asarray(inputs[f"nw{g}"], np.float32) for g in range(5)]
    nbs = [np.asarray(inputs[f"nb{g}"], np.float32) for g in range(5)]
    fbs = [np.asarray(inputs[f"fb{g}"], np.float32) for g in range(5)]

    cst = np.zeros((128, NCONST), np.float32)

    def put_bf16(name, arr128x128):
        u16 = np.ascontiguousarray(arr128x128.astype(bf16)).view(np.uint16)
        u32 = u16[:, 0::2].astype(np.uint32) | (
            u16[:, 1::2].astype(np.uint32) << 16)
        cst[:, CONST_OFF[name]:CONST_OFF[name] + 64] = u32.view(np.float32)

    put_bf16("ident", np.eye(128, dtype=np.float32))
    c1t = np.zeros((C, NB), np.float32)
    c2t = np.zeros((C, NB), np.float32)
    for j, (q0, w, g, i) in enumerate(BANDS):
        c = CHUNK_OF[j]
        r0 = q0 - CHUNKS[c][0]
        fw, nw, nb, fb = fws[g][i], nws[g][i], nbs[g][i], fbs[g][i]
        fwn = np.zeros((128, C), np.float32)
        fwn[r0:r0 + w, :] = (fw * nw[None, :]).T
        put_bf16(f"fwn{j}", fwn)
        c1t[:, j] = fb + fw @ nb
        c2t[:, j] = fw @ nw
    for c, (q0, rows, bl) in enumerate(CHUNKS):
        ind = np.zeros((128, NB), np.float32)
        for j in bl:
            qb, w, _g, _i = BANDS[j]
            ind[qb - q0:qb - q0 + w, j] = 1.0
        cst[:, CONST_OFF[f"ind{c}"]:CONST_OFF[f"ind{c}"] + NB] = ind
    cst[:, CONST_OFF["c1t"]:CONST_OFF["c1t"] + NB] = c1t
    cst[:, CONST_OFF["c2t"]:CONST_OFF["c2t"] + NB] = c2t
    cst[0:NB, CONST_OFF["ones34"]:CONST_OFF["ones34"] + C] = 1.0
    cst[0:NB, CONST_OFF["eye34"]:CONST_OFF["eye34"] + NB] = np.eye(NB)
    cst[0:NB, CONST_OFF["invc"]] = [1.0 / w for (_q0, w, _g, _i) in BANDS]
    cst[0:NB, CONST_OFF["epsv"]] = EPS
    return {"cst": cst}


def build_module(phases=4, ntc_cap=99, skip_out_dma=False, out_mode="mix"):
    import concourse.bacc as bacc
    import concourse.tile as tile
    import concourse.mybir as mybir
    from contextlib import ExitStack

    f32 = mybir.dt.float32
    bf16 = mybir.dt.bfloat16
    AF = mybir.ActivationFunctionType
    ALU = mybir.AluOpType
    nc = bacc.Bacc(None)

    x_d = nc.declare_dram_parameter("x", [T, Q], bf16, isOutput=False)
    cst_d = nc.declare_dram_parameter("cst", [128, NCONST], f32, isOutput=False)
    out_d = nc.declare_dram_parameter("out", [C, T, NB], f32, isOutput=True)

    with tile.TileContext(nc) as tc, ExitStack() as ctx:
        cpool = ctx.enter_context(tc.tile_pool(name="cpool", bufs=1))
        ldpool = ctx.enter_context(tc.tile_pool(name="ld", bufs=2))
        stpool = ctx.enter_context(tc.tile_pool(name="st", bufs=2))
        smpool = ctx.enter_context(tc.tile_pool(name="sm", bufs=4))
        # PSUM: 8 banks of 2KB, bank-granular tiles. 4 transpose banks
        # (4 chunks x 2 t-tiles packed per bank), 3 output banks, 1 small.
        ps_tp = ctx.enter_context(tc.tile_pool(name="ps_tp", bufs=4, space="PSUM"))
        ps_out = ctx.enter_context(tc.tile_pool(name="ps_out", bufs=3, space="PSUM"))
        ps_sm = ctx.enter_context(tc.tile_pool(name="ps_sm", bufs=1, space="PSUM"))

        # ident first (transposes need it); the bulk lands after the x DMAs
        cst = cpool.tile([128, NCONST], f32, tag="cst", name="cst_t")
        nc.sync.dma_start(cst[:, 0:64], cst_d[:, 0:64])

        def cview(name, cols, dt=f32, parts=128):
            o = CONST_OFF[name]
            v = cst[0:parts, o:o + (cols if dt is f32 else cols // 2)]
            return v.bitcast(dt) if dt is not f32 else v

        ident = cview("ident", 128, bf16)
        fwp = [cview(f"fwn{j}", 128, bf16) for j in range(NB)]
        ind = [cview(f"ind{c}", NB) for c in range(NCH)]
        c1t = cview("c1t", NB)
        c2t = cview("c2t", NB)
        ones34 = cview("ones34", C, parts=NB)
        eye34 = cview("eye34", NB, parts=NB)
        invc = cview("invc", 1, parts=NB)
        epsap = cview("epsv", 1, parts=NB)

        xT = [cpool.tile([CHUNKS[c][1], T], bf16, tag=f"xT{c}", name=f"xT{c}")
              for c in range(NCH)]
        musig = cpool.tile([NB, 2], f32, tag="musig", name="musig")
        bcast = cpool.tile([C, 2 * NB], f32, tag="bcast", name="bcast")
        btot = cpool.tile([C, NB], f32, tag="btot", name="btot")

        # preload both ACT tables used later (Sqrt for stats, Identity for
        # output copies) so the loads sit off the critical path
        warm = smpool.tile([1, 2], f32, tag="warm", name="warm")
        nc.vector.memset(warm[:], 0.0)
        nc.scalar.activation(warm[:, 0:1], warm[:, 0:1], AF.Sqrt,
                             bias=warm[:, 1:2], scale=1.0)
        nc.scalar.activation(warm[:, 0:1], warm[:, 0:1], AF.Identity,
                             bias=warm[:, 1:2], scale=1.0)

        # ---- L: load + transpose (+ windowed stats) ----
        CGRP = [[0, 1, 2, 3, 4], [5, 6, 7, 8, 9]]
        st6 = [smpool.tile([CHUNKS[c][1], 6 * len(WINS)], f32,
                           tag=f"st6_{c}", name=f"st6_{c}", bufs=1)
               for c in range(NCH)]
        tp = {}
        nat = None
        ld_starts = np.cumsum([0] + LDBS[:-1]).tolist()
        nat_off = 0
        for tt in range(NLD):
            blk, half = divmod(tt, TBLK)
            if tt in ld_starts:
                ldb = LDBS[ld_starts.index(tt)]
                nat_off = tt
                nat = ldpool.tile([TT, ldb, Q], bf16, tag="nat",
                                  name=f"nat{tt}")
                xv = x_d.rearrange("(b t) q -> t b q", t=TT)
                nc.sync.dma_start(nat[:], xv[:, tt:tt + ldb, :])
                if tt == 0:
                    nc.sync.dma_start(cst[:, 64:], cst_d[:, 64:])
                # tiny PE matmul absorbs the DMA-queue wait so transposes
                # carry at most one wait each
                dmy = ps_sm.tile([1, 1], f32, tag="small", name=f"dmy{tt}")
                nc.tensor.matmul(dmy[:], nat[0:1, 0, 0:1], nat[0:1, 0, 0:1],
                                 start=True, stop=True)
            if half == 0:
                for g, grp in enumerate(CGRP):
                    tp[g] = ps_tp.tile([128, len(grp) * TBLK * TTP], bf16,
                                       tag="tp", name=f"tp{blk}_{g}")
            for c in range(NCH):
                q0, rows, _bl = CHUNKS[c]
                g, idx = divmod(c, 5)
                col = (idx * TBLK + half) * TTP
                nc.tensor.transpose(tp[g][0:rows, col:col + TT],
                                    nat[:, tt - nat_off, q0:q0 + rows],
                                    ident[0:TT, 0:TT])
            if half == TBLK - 1:
                for c in range(NCH):
                    rows = CHUNKS[c][1]
                    g, idx = divmod(c, 5)
                    tpv = tp[g].rearrange("p (s t) -> p s t", t=TTP)
                    src = tpv[0:rows, idx * TBLK:(idx + 1) * TBLK, 0:TT]
                    dst = xT[c][:, blk * TBLK * TT:(blk + 1) * TBLK * TT
                                ].rearrange("p (s t) -> p s t", t=TT)
                    # DVE carries bn_stats too; give it fewer copies
                    if (c + blk) % 10 < 2:
                        nc.vector.tensor_copy(dst, src)
                    else:
                        nc.scalar.copy(dst, src)
            if phases >= 2:
                done_t = (tt + 1) * TT if half == TBLK - 1 else 0
                for wi, (ws, wl) in enumerate(WINS):
                    if done_t and done_t - TBLK * TT < ws + wl <= done_t:
                        assert ws % 2 == 0
                        for c in range(NCH):
                            rows = CHUNKS[c][1]
                            nc.vector.bn_stats(st6[c][:, 6 * wi:6 * (wi + 1)],
                                               xT[c][0:rows, ws:ws + wl])

        if phases == 1:
            nc.sync.dma_start(out_d[0:CHUNKS[0][1], 0:T // 2, 0],
                              xT[0][:].bitcast(f32))

        if phases >= 2:
            # ---- S: stats -> mu, rstd -> broadcast + btot ----
            stats_ps = ps_sm.tile([NB, 2], f32, tag="small", name="stats_ps")
            for c in range(NCH):
                rows = CHUNKS[c][1]
                s12 = smpool.tile([rows, 2], f32, tag="s12", name=f"s12_{c}")
                tmp = smpool.tile([rows, 1], f32, tag="tmp", name=f"tmp{c}")
                nc.vector.bn_aggr(s12[:], st6[c][:])
                nc.vector.tensor_mul(tmp[:], s12[:, 0:1], s12[:, 0:1])
                nc.vector.tensor_add(s12[:, 1:2], s12[:, 1:2], tmp[:])
                nc.tensor.matmul(stats_ps[:], ind[c][0:rows, :], s12[:],
                                 start=(c == 0), stop=(c == NCH - 1))

            # musig[:,0]=mu, musig[:,1]=rstd  (invc = 1/w, sums are of
            # per-row means/second-moments so /w gives band stats)
            ex2 = smpool.tile([NB, 1], f32, tag="ex2", name="ex2")
            var_t = smpool.tile([NB, 1], f32, tag="var", name="var_t")
            std_t = smpool.tile([NB, 1], f32, tag="std", name="std_t")
            nc.vector.tensor_scalar_mul(musig[:, 0:1], stats_ps[:, 0:1],
                                        invc[:])
            nc.vector.tensor_scalar_mul(ex2[:], stats_ps[:, 1:2], invc[:])
            nc.vector.tensor_mul(var_t[:], musig[:, 0:1], musig[:, 0:1])
            nc.vector.tensor_sub(var_t[:], ex2[:], var_t[:])
            nc.scalar.activation(std_t[:], var_t[:], AF.Sqrt, bias=epsap[:],
                                 scale=1.0)
            nc.vector.reciprocal(musig[:, 1:2], std_t[:])

            # diag trick: [diag(rstd) | diag(mu*rstd)], then ones34^T @ .
            mrs = smpool.tile([NB, 1], f32, tag="mrs", name="mrs")
            nc.vector.tensor_mul(mrs[:], musig[:, 0:1], musig[:, 1:2])
            dg = smpool.tile([NB, 2 * NB], f32, tag="dg", name="dg")
            nc.vector.tensor_scalar_mul(dg[:, 0:NB], eye34[:], musig[:, 1:2])
            nc.vector.tensor_scalar_mul(dg[:, NB:2 * NB], eye34[:], mrs[:])
            bc_ps = ps_sm.tile([C, 2 * NB], f32, tag="small", name="bc_ps")
            nc.tensor.matmul(bc_ps[:], ones34[:], dg[:], start=True, stop=True)
            nc.vector.tensor_copy(bcast[:], bc_ps[:])
            nc.vector.tensor_mul(btot[:], bcast[:, NB:2 * NB], c2t[:])
            nc.vector.tensor_sub(btot[:], c1t[:], btot[:])

        if phases == 2:
            nc.sync.dma_start(out_d[0:NB, 0, 0:2], musig[:])
            nc.sync.dma_start(out_d[0:C, 1, 0:NB], btot[:])

        if phases >= 3:
            # ---- O: per (t-chunk, band) matmul + fused scale/bias copy ----
            t0 = 0
            for tk, TC in enumerate(TCS[:min(len(TCS), ntc_cap)]):
                stag = stpool.tile([C, max(TCS) * NB], f32, tag="stag",
                                   name=f"stag{tk}")
                sv = stag.rearrange("p (t j) -> p t j", j=NB)
                for j in range(NB):
                    c, base, K = _band_window(j)
                    ops = ps_out.tile([C, TC], f32, tag="outp",
                                      name=f"ops{tk}_{j}")
                    nc.tensor.matmul(ops[:], fwp[j][base:base + K, :],
                                     xT[c][base:base + K, t0:t0 + TC],
                                     start=True, stop=True)
                    use_act = ((j + tk) % 2 < 1 if tk == 0
                               else (j + tk) % 9 < 5)
                    if out_mode == "plain":
                        (nc.scalar.copy if use_act else nc.vector.tensor_copy)(
                            sv[:, 0:TC, j], ops[:])
                    elif out_mode == "act" or (out_mode == "mix" and use_act):
                        nc.scalar.activation(sv[:, 0:TC, j], ops[:],
                                             AF.Identity,
                                             bias=btot[:, j:j + 1],
                                             scale=bcast[:, j:j + 1])
                    else:
                        nc.vector.tensor_scalar(sv[:, 0:TC, j], ops[:],
                                                bcast[:, j:j + 1],
                                                btot[:, j:j + 1],
                                                ALU.mult, ALU.add)
                if not skip_out_dma:
                    nc.sync.dma_start(out_d[:, t0:t0 + TC, :],
                                      sv[:, 0:TC, :])
                t0 += TC

    _finalize(nc)
    return nc


def _finalize(nc):
    import concourse.mybir as mybir
    nc.compile()
    # compile()'s late passes can leave >1-wait instructions, which walrus
    # rejects for some instruction types and hardware mishandles for others.
    nc.generate_event_semaphores()
    nc.codegen_inst_isa_subclasses()
    m2 = mybir.parse_bytes(nc.to_json_bytes())
    for fn in m2.functions:
        for bb in fn.blocks:
            for i in bb.instructions:
                si = i.sync_info
                n = len(si.on_wait) if si and si.on_wait else 0
                assert n <= 1 or type(i).__name__ == "InstEventSemaphore", (
                    f"multi-wait survived: {i.name} {type(i).__name__} {n}")


_CACHE = {}


def _get_module():
    if "nc" not in _CACHE:
        _CACHE["nc"] = build_module()
    return _CACHE["nc"]


def prepare_in_maps(inputs):
    import concourse.mybir as mybir
    bf16 = mybir.dt.np(mybir.dt.bfloat16)
    x = np.ascontiguousarray(
        np.asarray(inputs["x"], dtype=np.float32)).reshape(B, T, Q).astype(bf16)
    base = host_constants(inputs)
    return [dict(base, x=x[i]) for i in range(B)]


def kernel(**inputs):
    from concourse.bass_utils import run_bass_kernel_spmd

    nc = _get_module()
    in_maps = prepare_in_maps(inputs)
    res = run_bass_kernel_spmd(nc, in_maps, core_ids=list(range(B)))
    return np.stack([res.results[i]["out"] for i in range(B)], axis=0)
